# revision 20
# baseline (speedup 1.0000x reference)
"""Trainium2 Bass kernel for BigReimplementationPathIntegrator.

Data-parallel over 8 NeuronCores: each core encodes 16 images with the
3-conv + 3-FC encoder, then runs the 100-step GRU locally on its batch
shard. Weights are replicated; batch dims of images/actions are sharded.

Layout strategy per core (B=16):
- convs as im2col matmuls (out channels on PSUM partitions). The stride-3
  convs are fed by "v-major" reshuffles (DVE copies) so im2col DMAs move
  contiguous runs.
- FC layers keep batch (16) on PSUM partitions; activations transposed
  back to feature-on-partition via PE transposes for the next layer.
- GRU: gates in psum[80, 512] (gate blocks r/z/n at partitions 0/32/64),
  hidden state kept both as h [16, 512] (for elementwise) and
  hT [128, 4*16] (feature-on-partition, matmul stationary operand).
  Input projections gi for all steps are precomputed on-device into DRAM
  and streamed back per step.
"""

import os
import numpy as np

import concourse.bass as bass
import concourse.bacc as bacc
import concourse.tile as tile
from concourse import mybir
from concourse import bass_utils

F32 = mybir.dt.float32
AF = mybir.ActivationFunctionType

NCORES = 8
B_FULL = 128
BC = B_FULL // NCORES            # 16 images per core
T_STEPS = int(os.environ.get("BASS_GRU_T", "100"))
SKIP_CNN = bool(int(os.environ.get("BASS_SKIP_CNN", "0")))
SKIP_GRU = bool(int(os.environ.get("BASS_SKIP_GRU", "0")))
SKIP_GI = bool(int(os.environ.get("BASS_SKIP_GI", "0")))
CNN_STAGES = int(os.environ.get("BASS_CNN_STAGES", "9"))
HID = 512
G3 = 3 * HID                     # 1536

C1_IN, C1_OUT = 72, 24
C2_IN, C2_OUT = 24, 9
C3_IN, C3_OUT = 9, 4
K1 = 3 * 5 * 5                   # 75
N1 = BC * C1_OUT * C1_OUT        # 9216
N2 = BC * C2_OUT * C2_OUT        # 1296
N3 = BC * C3_OUT * C3_OUT        # 256


def _ceil_div(a, b):
    return -(-a // b)


def _conv_valid_range(k, n_out, n_in):
    """valid output range [lo, hi] for y = 3*o + k - 2 in [0, n_in)."""
    lo = max(0, _ceil_div(2 - k, 3))
    hi = min(n_out - 1, (n_in - 1 + 2 - k) // 3)
    return lo, hi


def _vmaj(k):
    """y = 3u + v decomposition of y = 3o + k - 2: u = o + delta."""
    v = (k - 2) % 3
    delta = (k - 2 - v) // 3
    return v, delta


class _Pools:
    """Manually ordered pool lifetimes (non-LIFO open/close)."""

    def __init__(self, tc):
        self.tc = tc
        self.cms = {}

    def open(self, name, **kw):
        cm = self.tc.tile_pool(name=name, **kw)
        pool = cm.__enter__()
        self.cms[name] = cm
        return pool

    def close(self, name):
        self.cms.pop(name).__exit__(None, None, None)

    def close_all(self):
        for name in reversed(list(self.cms)):
            self.close(name)


def build_program():
    nc = bacc.Bacc("TRN2", target_bir_lowering=False, debug=False)

    def din(name, shape):
        return nc.dram_tensor(name, shape, F32, kind="ExternalInput").ap()

    t_in = dict(
        images=din("images", [BC, 3, C1_IN, C1_IN]),
        aT=din("aT", [3, T_STEPS * BC]),
        cw1m=din("cw1m", [K1, 64]),
        cb1=din("cb1", [64, 1]),
        cw2p=din("cw2p", [128, 5 * 64]),
        cb2=din("cb2", [64, 1]),
        cw3p=din("cw3p", [128, 5 * 64]),
        cb3=din("cb3", [64, 1]),
        fw1tp=din("fw1tp", [128, 8 * 1024]),
        fb1=din("fb1", [1, 1024]),
        fw2tp=din("fw2tp", [128, 8 * 512]),
        fb2=din("fb2", [1, 512]),
        fw3tp=din("fw3tp", [128, 4 * 512]),
        fb3=din("fb3", [1, 512]),
        whhTp=din("whhTp", [128, 4 * G3]),
        wip=din("wip", [3, G3]),
        bhn=din("bhn", [1, HID]),
        id16=din("id16", [16, 16]),
        outs=nc.dram_tensor("outs", [BC, T_STEPS, HID], F32,
                            kind="ExternalOutput").ap(),
        hT_out=nc.dram_tensor("hT", [BC, HID], F32, kind="ExternalOutput").ap(),
    )

    with tile.TileContext(nc) as tc:
        _build_tile(nc, tc, t_in)
    nc.compile()
    return nc


def _build_tile(nc, tc, t):
    P = _Pools(tc)
    singles = P.open("singles", bufs=1)

    def load(ap_in, shape, name):
        tl = singles.tile(shape, F32, tag=name)
        nc.sync.dma_start(out=tl, in_=ap_in)
        return tl

    cw1_sb = load(t["cw1m"], [K1, 64], "cw1")
    cb1_sb = load(t["cb1"], [64, 1], "cb1")
    cw2_sb = load(t["cw2p"], [128, 5 * 64], "cw2")
    cb2_sb = load(t["cb2"], [64, 1], "cb2")
    cw3_sb = load(t["cw3p"], [128, 5 * 64], "cw3")
    cb3_sb = load(t["cb3"], [64, 1], "cb3")
    fb1_sb = load(t["fb1"], [1, 1024], "fb1")
    fw2_sb = load(t["fw2tp"], [128, 8 * 512], "fw2")
    fb2_sb = load(t["fb2"], [1, 512], "fb2")
    fw3_sb = load(t["fw3tp"], [128, 4 * 512], "fw3")
    fb3_sb = load(t["fb3"], [1, 512], "fb3")
    whh_sb = load(t["whhTp"], [128, 4 * G3], "whh")
    wip_sb = load(t["wip"], [3, G3], "wip")
    bhn_sb = load(t["bhn"], [1, HID], "bhn")
    id16_sb = load(t["id16"], [16, 16], "id16")
    aT_sb = load(t["aT"], [3, T_STEPS * BC], "aT")
    ones16 = singles.tile([1, 16], F32, tag="ones16")
    nc.vector.memset(ones16, 1.0)

    trp = P.open("trp", bufs=2, space="PSUM")
    hpool = P.open("hpool", bufs=2)
    htpool = P.open("htpool", bufs=2)
    dram = P.open("dram", bufs=1, space="DRAM")
    gi_dram = dram.tile([T_STEPS, 48, HID], F32)

    # ---------------- gi precompute: gi[(t,b)] = [a0 a1 1] @ wip --------
    if SKIP_GI:
        nrow = 0
    gipsum = P.open("gipsum", bufs=2, space="PSUM")
    gisb = P.open("gisb", bufs=3)
    nrow = 0 if SKIP_GI else T_STEPS * BC
    for c in range(_ceil_div(nrow, 128)):
        r0 = c * 128
        cnt = min(128, nrow - r0)
        nst = cnt // BC
        for j in range(3):
            gp = gipsum.tile([128, HID], F32, tag="gp")
            nc.tensor.matmul(
                gp[:cnt, :], lhsT=aT_sb[:, r0:r0 + cnt],
                rhs=wip_sb[:, j * HID:(j + 1) * HID], start=True, stop=True)
            gs = gisb.tile([128, HID], F32, tag="gs")
            nc.any.tensor_copy(gs[:cnt, :], gp[:cnt, :])
            for ts in range(nst):
                nc.sync.dma_start(
                    out=gi_dram[c * 8 + ts, j * BC:(j + 1) * BC, :],
                    in_=gs[ts * BC:(ts + 1) * BC, :])
    P.close("gipsum")
    P.close("gisb")

    if SKIP_CNN:
        h_cur = hpool.tile([16, HID], F32, tag="h", name="h0")
        nc.vector.memset(h_cur, 0.01)
        hT_cur = htpool.tile([128, 4 * 16], F32, tag="hT", name="h0T")
        nc.vector.memset(hT_cur, 0.01)
        return _gru_loop(nc, tc, t, P, locals())

    # ---------------- CNN encoder ----------------
    # One rotating 2-slot pool holds the stage chain: each stage only needs
    # itself + its producer alive.
    cpsum = P.open("cpsum", bufs=2, space="PSUM")
    chain = P.open("chain", bufs=2)

    def stage(shape):
        return chain.tile(shape, F32, tag="stage", name="stage")

    imgs_sb = stage([BC * 3, C1_IN * C1_IN])
    nc.sync.dma_start(out=imgs_sb, in_=t["images"].rearrange("b c y x -> c b (y x)"))
    # v-major image with u-axes padded to 25 (u index = o + delta + 1 covers
    # u in [-1, 23]; out-of-range pixels are the conv zero-padding)
    img_pad = stage([BC * 3, 9 * 25 * 25])
    nc.vector.memset(img_pad, 0.0)
    src5 = imgs_sb.rearrange(
        "p (uy vy ux vx) -> p vy vx uy ux", vy=3, vx=3, uy=24, ux=24)
    dst5 = img_pad.rearrange(
        "p (vy vx uy ux) -> p vy vx uy ux", vy=3, vx=3, uy=25, ux=25)
    for vy in range(3):
        for vx in range(3):
            nc.vector.tensor_copy(dst5[:, vy, vx, 1:25, 1:25], src5[:, vy, vx])

    patches1 = stage([K1, N1])
    pat1 = patches1.rearrange(
        "k (b oy ox) -> k b oy ox", b=BC, oy=C1_OUT, ox=C1_OUT)
    srcv = img_pad.rearrange(
        "(c b) (vy vx uy ux) -> c b vy vx uy ux",
        c=3, vy=3, vx=3, uy=25, ux=25)
    for ci in range(3):
        for ky in range(5):
            vy, dy = _vmaj(ky)
            for kx in range(5):
                vx, dx = _vmaj(kx)
                row = ci * 25 + ky * 5 + kx
                nc.sync.dma_start(
                    out=pat1[row:row + 1].opt(),
                    in_=srcv[ci, :, vy, vx,
                             dy + 1:dy + 25, dx + 1:dx + 25])

    x1_sb = stage([64, N1])
    for n in range(N1 // 512):
        ps = cpsum.tile([64, 512], F32, tag="cps")
        nc.tensor.matmul(
            ps, lhsT=cw1_sb, rhs=patches1[:, n * 512:(n + 1) * 512],
            start=True, stop=True)
        nc.scalar.activation(
            x1_sb[:, n * 512:(n + 1) * 512], ps, AF.Relu, bias=cb1_sb)

    if CNN_STAGES < 2:
        h_cur = hpool.tile([16, HID], F32, tag="h", name="h0")
        nc.vector.memset(h_cur, 0.01)
        hT_cur = htpool.tile([128, 4 * 16], F32, tag="hT", name="h0T")
        nc.vector.memset(hT_cur, 0.01)
        return _gru_loop(nc, tc, t, P, locals())

    # conv2
    x1_vu = stage([64, N1])
    s6 = x1_sb.rearrange(
        "p (b uy vy ux vx) -> p vy vx b uy ux", b=BC, uy=8, vy=3, ux=8, vx=3)
    d6 = x1_vu.rearrange(
        "p (vy vx uy ux b) -> p vy vx uy ux b", b=BC, vy=3, vx=3, uy=8, ux=8)
    for vy in range(3):
        for vx in range(3):
            nc.vector.tensor_copy(
                d6[:, vy, vx], s6[:, vy, vx].transpose([0, 2, 3, 1]))

    patches2 = stage([128, 5 * N2])
    nc.vector.memset(patches2, 0.0)
    pat2 = patches2.rearrange(
        "p (c oy ox b) -> p c oy ox b", c=5, b=BC, oy=C2_OUT, ox=C2_OUT)
    for ky in range(3):
        vy, dy = _vmaj(ky)
        ylo, yhi = _conv_valid_range(ky, C2_OUT, C2_IN)
        for kx in range(3):
            vx, dx = _vmaj(kx)
            xlo, xhi = _conv_valid_range(kx, C2_OUT, C2_IN)
            kp = ky * 3 + kx
            nc.sync.dma_start(
                out=pat2[(kp % 2) * 64:(kp % 2) * 64 + 64, kp // 2,
                         ylo:yhi + 1, xlo:xhi + 1, :].opt(),
                in_=d6[:, vy, vx].opt())

    x2_sb = stage([64, N2])
    nsz = [512, 512, N2 - 1024]
    for n in range(3):
        n0 = n * 512
        ps = cpsum.tile([64, 512], F32, tag="cps")
        for c in range(5):
            nc.tensor.matmul(
                ps[:, :nsz[n]], lhsT=cw2_sb[:, c * 64:(c + 1) * 64],
                rhs=pat2[:, c].rearrange("p oy ox b -> p (oy ox b)")[
                    :, n0:n0 + nsz[n]],
                start=(c == 0), stop=(c == 4))
        nc.scalar.activation(
            x2_sb[:, n0:n0 + nsz[n]], ps[:, :nsz[n]], AF.Relu, bias=cb2_sb)

    if CNN_STAGES < 3:
        h_cur = hpool.tile([16, HID], F32, tag="h", name="h0")
        nc.vector.memset(h_cur, 0.01)
        hT_cur = htpool.tile([128, 4 * 16], F32, tag="hT", name="h0T")
        nc.vector.memset(hT_cur, 0.01)
        return _gru_loop(nc, tc, t, P, locals())

    # conv3
    x2_vu = stage([64, N2])
    s7 = x2_sb.rearrange(
        "p (uy vy ux vx b) -> p vy vx uy ux b", b=BC, uy=3, vy=3, ux=3, vx=3)
    d7 = x2_vu.rearrange(
        "p (vy vx uy ux b) -> p vy vx uy ux b", b=BC, vy=3, vx=3, uy=3, ux=3)
    for vy in range(3):
        for vx in range(3):
            nc.vector.tensor_copy(d7[:, vy, vx], s7[:, vy, vx])

    patches3 = stage([128, 5 * N3])
    nc.vector.memset(patches3, 0.0)
    pat3 = patches3.rearrange(
        "p (c oy ox b) -> p c oy ox b", c=5, b=BC, oy=C3_OUT, ox=C3_OUT)
    for ky in range(3):
        vy, dy = _vmaj(ky)
        ylo, yhi = _conv_valid_range(ky, C3_OUT, C3_IN)
        for kx in range(3):
            vx, dx = _vmaj(kx)
            xlo, xhi = _conv_valid_range(kx, C3_OUT, C3_IN)
            kp = ky * 3 + kx
            nc.sync.dma_start(
                out=pat3[(kp % 2) * 64:(kp % 2) * 64 + 64, kp // 2,
                         ylo:yhi + 1, xlo:xhi + 1, :].opt(),
                in_=d7[:, vy, vx].opt())

    x3_sb = stage([64, N3])
    ps3 = cpsum.tile([64, N3], F32, tag="cps")
    for c in range(5):
        nc.tensor.matmul(
            ps3, lhsT=cw3_sb[:, c * 64:(c + 1) * 64],
            rhs=pat3[:, c].rearrange("p oy ox b -> p (oy ox b)"),
            start=(c == 0), stop=(c == 4))
    nc.scalar.activation(x3_sb, ps3, AF.Relu, bias=cb3_sb)

    if CNN_STAGES < 4:
        h_cur = hpool.tile([16, HID], F32, tag="h", name="h0")
        nc.vector.memset(h_cur, 0.01)
        hT_cur = htpool.tile([128, 4 * 16], F32, tag="hT", name="h0T")
        nc.vector.memset(hT_cur, 0.01)
        return _gru_loop(nc, tc, t, P, locals())

    # xT: partitions f' = s*64+co (mod 128), col-blocks q = f'//128
    xT = stage([128, 8 * BC])
    x3v = x3_sb.rearrange("p (s b) -> p s b", b=BC, s=16)
    xTv = xT.rearrange("(sl p) (q b) -> sl p q b", sl=2, q=8, b=BC)
    for s in range(16):
        nc.sync.dma_start(out=xTv[s % 2, :, s // 2, :], in_=x3v[:, s, :])

    fw1_sb = stage([128, 8 * 1024])
    nc.sync.dma_start(out=fw1_sb, in_=t["fw1tp"])

    if CNN_STAGES < 5:
        h_cur = hpool.tile([16, HID], F32, tag="h", name="h0")
        nc.vector.memset(h_cur, 0.01)
        hT_cur = htpool.tile([128, 4 * 16], F32, tag="hT", name="h0T")
        nc.vector.memset(hT_cur, 0.01)
        return _gru_loop(nc, tc, t, P, locals())

    # ---------------- FC stack ----------------
    fcact = P.open("fcact", bufs=1)
    fcT = P.open("fcT", bufs=1)

    def fc(xT_in, nq, w_sb, b_sb, nout, relu, out_pool, out_tag):
        o_sb = out_pool.tile([16, nout], F32, tag=out_tag)
        for n in range(nout // 512):
            ps = cpsum.tile([64, 512], F32, tag="cps")
            for q in range(nq):
                nc.tensor.matmul(
                    ps[:16, :], lhsT=xT_in[:, q * 16:(q + 1) * 16],
                    rhs=w_sb[:, q * nout + n * 512:q * nout + (n + 1) * 512],
                    start=(q == 0), stop=False)
            nc.tensor.matmul(
                ps[:16, :], lhsT=ones16, rhs=b_sb[:, n * 512:(n + 1) * 512],
                start=False, stop=True)
            nc.scalar.activation(
                o_sb[:, n * 512:(n + 1) * 512], ps[:16, :],
                AF.Relu if relu else AF.Copy)
        return o_sb

    def transpose_to(h_sb, nchunk, pool, tag):
        hT = pool.tile([128, nchunk * 16], F32, tag=tag)
        for c in range(nchunk):
            tr = trp.tile([128, 16], F32, tag="tr")
            nc.tensor.transpose(tr, h_sb[:, c * 128:(c + 1) * 128], id16_sb)
            if c % 2 == 0:
                nc.vector.tensor_copy(hT[:, c * 16:(c + 1) * 16], tr)
            else:
                nc.scalar.activation(hT[:, c * 16:(c + 1) * 16], tr, AF.Copy)
        return hT

    o1 = fc(xT, 8, fw1_sb, fb1_sb, 1024, True, fcact, "o1")
    o1T = transpose_to(o1, 8, fcT, "o1T")
    o2 = fc(o1T, 8, fw2_sb, fb2_sb, 512, True, fcact, "o2")
    o2T = transpose_to(o2, 4, fcT, "o2T")
    h_cur = fc(o2T, 4, fw3_sb, fb3_sb, 512, False, hpool, "h")
    hT_cur = transpose_to(h_cur, 4, htpool, "hT")
    P.close("fcT")
    P.close("fcact")
    P.close("chain")
    P.close("cpsum")

    return _gru_loop(nc, tc, t, P, locals())


def _gru_loop(nc, tc, t, P, env):
    gi_dram = env["gi_dram"]
    whh_sb, bhn_sb, ones16, id16_sb = (
        env["whh_sb"], env["bhn_sb"], env["ones16"], env["id16_sb"])
    hpool, htpool, trp = env["hpool"], env["htpool"], env["trp"]
    h_cur, hT_cur = env["h_cur"], env["hT_cur"]
    if SKIP_GRU:
        nc.sync.dma_start(out=t["hT_out"], in_=h_cur)
        nc.sync.dma_start(out=t["outs"][:, 0, :], in_=h_cur)
        P.close_all()
        return

    # ---------------- GRU loop ----------------
    gpsum = P.open("gpsum", bufs=2, space="PSUM")
    gipool = P.open("gipool", bufs=6)
    ew = P.open("ew", bufs=3)
    outs, hT_out = t["outs"], t["hT_out"]
    for step in range(T_STEPS):
        gi_r = gipool.tile([16, HID], F32, tag="gi")
        nc.sync.dma_start(out=gi_r, in_=gi_dram[step, 0:BC, :])
        gi_z = gipool.tile([16, HID], F32, tag="gi")
        nc.sync.dma_start(out=gi_z, in_=gi_dram[step, BC:2 * BC, :])
        gi_n = gipool.tile([16, HID], F32, tag="gi")
        nc.sync.dma_start(out=gi_n, in_=gi_dram[step, 2 * BC:3 * BC, :])

        # gate blocks r/z/n at psum partitions 0/32/64 (PE needs 32-aligned)
        g = gpsum.tile([80, HID], F32, tag="g")
        for j in range(3):
            for c in range(4):
                nc.tensor.matmul(
                    g[32 * j:32 * j + 16, :],
                    lhsT=hT_cur[:, c * 16:(c + 1) * 16],
                    rhs=whh_sb[:, c * G3 + j * HID:c * G3 + (j + 1) * HID],
                    start=(c == 0), stop=(c == 3 and j < 2))
        nc.tensor.matmul(
            g[64:80, :], lhsT=ones16, rhs=bhn_sb, start=False, stop=True)

        sr = ew.tile([16, HID], F32, tag="sr")
        nc.vector.tensor_add(sr, g[0:16, :], gi_r)
        sz = ew.tile([16, HID], F32, tag="sz")
        nc.vector.tensor_add(sz, g[32:48, :], gi_z)
        r_t = ew.tile([16, HID], F32, tag="r")
        nc.scalar.activation(r_t, sr, AF.Sigmoid)
        z_t = ew.tile([16, HID], F32, tag="z")
        nc.scalar.activation(z_t, sz, AF.Sigmoid)
        t1 = ew.tile([16, HID], F32, tag="tmp")
        nc.vector.tensor_mul(t1, g[64:80, :], r_t)
        t2 = ew.tile([16, HID], F32, tag="tmp")
        nc.vector.tensor_add(t2, t1, gi_n)
        n_t = ew.tile([16, HID], F32, tag="n")
        nc.scalar.activation(n_t, t2, AF.Tanh)
        d_t = ew.tile([16, HID], F32, tag="tmp")
        nc.vector.tensor_sub(d_t, h_cur, n_t)
        e_t = ew.tile([16, HID], F32, tag="tmp")
        nc.vector.tensor_mul(e_t, z_t, d_t)
        h_new = hpool.tile([16, HID], F32, tag="h")
        nc.vector.tensor_add(h_new, n_t, e_t)

        nc.sync.dma_start(out=outs[:, step, :], in_=h_new)
        if step == T_STEPS - 1:
            nc.sync.dma_start(out=hT_out, in_=h_new)
            break

        hT_new = htpool.tile([128, 4 * 16], F32, tag="hT")
        for c in range(4):
            tr = trp.tile([128, 16], F32, tag="tr")
            nc.tensor.transpose(tr, h_new[:, c * 128:(c + 1) * 128], id16_sb)
            if c % 2 == 0:
                nc.vector.tensor_copy(hT_new[:, c * 16:(c + 1) * 16], tr)
            else:
                nc.scalar.activation(hT_new[:, c * 16:(c + 1) * 16], tr, AF.Copy)
        h_cur, hT_cur = h_new, hT_new

    P.close_all()


# ---------------------------------------------------------------------------
# host-side: input prep, sharding, run, gather
# ---------------------------------------------------------------------------

_PROGRAM_CACHE = {}


def _get_program():
    if "nc" not in _PROGRAM_CACHE:
        _PROGRAM_CACHE["nc"] = build_program()
    return _PROGRAM_CACHE["nc"]


def _prep_shared(cw1, cb1, cw2, cb2, cw3, cb3, fw1, fb1, fw2, fb2, fw3, fb3,
                 w_ih, w_hh, b_ih, b_hh):
    f = np.float32
    cw1m = np.ascontiguousarray(
        cw1.transpose(1, 2, 3, 0).reshape(K1, 64)).astype(f)

    def pack_conv(cw):
        # rows (ky,kx,ci) -> 5 col-blocks of 128-row tiles
        m = cw.transpose(2, 3, 1, 0).reshape(576, 64).astype(f)
        p = np.zeros((5 * 128, 64), f)
        p[:576] = m
        return np.ascontiguousarray(
            p.reshape(5, 128, 64).transpose(1, 0, 2).reshape(128, 5 * 64))

    # fc1 contraction reorder: f = co*16+s -> f' = s*64+co
    perm = np.arange(1024).reshape(64, 16).T.reshape(-1)
    fw1_perm = fw1[:, perm]

    def pack_fcT(w, nq):
        nout, nin = w.shape
        assert nin == nq * 128
        wt = w.T.reshape(nq, 128, nout).astype(f)
        return np.ascontiguousarray(
            wt.transpose(1, 0, 2).reshape(128, nq * nout))

    mask = np.concatenate([np.ones(1024, f), np.zeros(512, f)])
    wip = np.concatenate(
        [w_ih.T.astype(f), (b_ih + b_hh * mask)[None].astype(f)], axis=0)

    return {
        "cw1m": cw1m, "cb1": cb1.reshape(64, 1).astype(f),
        "cw2p": pack_conv(cw2), "cb2": cb2.reshape(64, 1).astype(f),
        "cw3p": pack_conv(cw3), "cb3": cb3.reshape(64, 1).astype(f),
        "fw1tp": pack_fcT(fw1_perm, 8), "fb1": fb1[None].astype(f),
        "fw2tp": pack_fcT(fw2, 8), "fb2": fb2[None].astype(f),
        "fw3tp": pack_fcT(fw3, 4), "fb3": fb3[None].astype(f),
        "whhTp": np.ascontiguousarray(
            w_hh.T.reshape(4, 128, G3).transpose(1, 0, 2).reshape(128, 4 * G3)
        ).astype(f),
        "wip": wip,
        "bhn": np.ascontiguousarray(b_hh[1024:])[None].astype(f),
        "id16": np.eye(16, dtype=f),
    }


def make_in_maps(images, actions, **weights):
    shared = _prep_shared(**weights)
    in_maps = []
    for i in range(NCORES):
        sl = slice(i * BC, (i + 1) * BC)
        img = np.ascontiguousarray(images[sl]).astype(np.float32)
        act = np.asarray(actions[sl][:, :T_STEPS]).astype(np.float32)
        A = np.concatenate(
            [act.transpose(1, 0, 2).reshape(T_STEPS * BC, 2),
             np.ones((T_STEPS * BC, 1), np.float32)], axis=1)
        m = dict(shared)
        m["images"] = img
        m["aT"] = np.ascontiguousarray(A.T)
        in_maps.append(m)
    return in_maps


def kernel(images, actions, cw1, cb1, cw2, cb2, cw3, cb3,
           fw1, fb1, fw2, fb2, fw3, fb3, w_ih, w_hh, b_ih, b_hh):
    args = {k: np.asarray(v) for k, v in locals().items()}
    nc = _get_program()
    in_maps = make_in_maps(**args)
    res = bass_utils.run_bass_kernel_spmd(nc, in_maps, core_ids=list(range(NCORES)))
    outs = np.concatenate([res.results[i]["outs"] for i in range(NCORES)], axis=0)
    hT = np.concatenate([res.results[i]["hT"] for i in range(NCORES)], axis=0)
    return outs, hT
